# revision 23
# baseline (speedup 1.0000x reference)
"""LCAConv (locally competitive algorithm, convolutional sparse coding) on
8 trn2 NeuronCores — data-parallel over batch (1 sample per core).

Math (matches the jax reference):
  x2   = standardize(x)                       per-sample, ddof=1, eps 1e-12
  b    = conv(x2, D, pad=3)                   input drive [32,64,64]
  u_1  = 0.01*b;  a_t = soft_threshold(u_{t-1}, 0.1)
  u_t  = u_{t-1} + 0.01*b - 0.01*clip(u_{t-1}, +-0.1) - 0.01*conv(a_t, G, pad=6)
  out  = a_10 = ST(u_9)   ->  8 Gram-conv iterations on device.

The Gram conv is factorized: conv(a, G) == conv(conv_T(a, D), D), cutting
tensor-engine work ~2.6x vs streaming the 13x13 Gram tensor directly.

Device mapping: 4x4 phase-packed layout. Pixel (y,x) = (4jy+ty, 4jx+tx).
u/a/b live as [128 = 8n x 16ph, 4 n-blocks x 16x16 j-grid]; each LCA
iteration is:
  conv1 (recon = D a):   9 tap-groups x 4 K-chunks, N=324  -> psum [48,18,18]
  R-stack: 2 scalar copies pack tap-pairs into K=96
  ident:  u += b01 - 0.01*clip(u)  (2 matmuls x 4 blocks)
  conv2 (u -= 0.01 D^T recon): 6 supergroups x 4 blocks, N=256
u is accumulated and kept resident in PSUM across all iterations. Matmuls
run in float32r (1 col/cycle, ~1e-4 operand rounding).
"""
import os
import sys
import types
import numpy as np

# ---------------------------------------------------------------- constants
NN, IC, K7 = 32, 3, 7                  # neurons, in-channels, kernel
H = W = 64
P = 4                                  # phase packing
J = H // P                             # 16 j-grid
NB = 4                                 # n-blocks (32/8)
NSUB = NN // NB                        # 8 neurons per block
PH = P * P                             # 16 phases
THRESH, TAU, LCA_ITERS = 0.1, 100.0, 10
ITERS_DEV = LCA_ITERS - 2              # first iter folded into b, last is ST only
B = 8                                  # batch == n cores

_CACHE = {}


# ------------------------------------------------------------- host packing
def _pack_w1(D2):
    """conv1 (recon = D a) lhsT tiles [9, NB, 128, 48]."""
    W1 = np.zeros((9, NB, 128, 48), np.float64)
    ns_, sy_, sx_ = np.meshgrid(np.arange(NSUB), np.arange(P), np.arange(P),
                                indexing='ij')
    c_, ty_, tx_ = np.meshgrid(np.arange(IC), np.arange(P), np.arange(P),
                               indexing='ij')
    for gy in (-1, 0, 1):
        for gx in (-1, 0, 1):
            g = (gy + 1) * 3 + (gx + 1)
            for nb in range(NB):
                ky = ty_.reshape(1, -1) - sy_.reshape(-1, 1) - 4 * gy + 3
                kx = tx_.reshape(1, -1) - sx_.reshape(-1, 1) - 4 * gx + 3
                ok = (ky >= 0) & (ky < 7) & (kx >= 0) & (kx < 7)
                n_full = (nb * NSUB + ns_).reshape(-1, 1)
                c_full = c_.reshape(1, -1)
                vals = D2[n_full, c_full, np.clip(ky, 0, 6), np.clip(kx, 0, 6)]
                W1[g, nb] = np.where(ok, vals, 0.0)
    return W1


def _pack_w2(D2, scale):
    """conv2 / b-conv lhsT tiles [9, NB, 48, 128], value scale*D2."""
    W2 = np.zeros((9, NB, 48, 128), np.float64)
    c_, sy_, sx_ = np.meshgrid(np.arange(IC), np.arange(P), np.arange(P),
                               indexing='ij')
    ns_, ty_, tx_ = np.meshgrid(np.arange(NSUB), np.arange(P), np.arange(P),
                                indexing='ij')
    for gy in (-1, 0, 1):
        for gx in (-1, 0, 1):
            g = (gy + 1) * 3 + (gx + 1)
            for nb in range(NB):
                ky = 4 * gy + sy_.reshape(-1, 1) - ty_.reshape(1, -1) + 3
                kx = 4 * gx + sx_.reshape(-1, 1) - tx_.reshape(1, -1) + 3
                ok = (ky >= 0) & (ky < 7) & (kx >= 0) & (kx < 7)
                n_full = (nb * NSUB + ns_).reshape(1, -1)
                c_full = c_.reshape(-1, 1)
                vals = D2[n_full, c_full, np.clip(ky, 0, 6), np.clip(kx, 0, 6)]
                W2[g, nb] = np.where(ok, scale * vals, 0.0)
    return W2


def _host_pack(D):
    """Everything derived from D only (weights)."""
    D2 = np.asarray(D, np.float64).reshape(NN, IC, K7, K7)
    W1 = _pack_w1(D2)                                      # [9,4,128,48]
    W2 = _pack_w2(D2, -0.01)                               # [9,4,48,128]

    # conv1 lhsT, fp8 DoubleRow: [128, g, pair, member, 48], nb = pair*2+member
    W1D = np.ascontiguousarray(
        W1.reshape(9, 2, 2, 128, 48).transpose(3, 0, 1, 2, 4))

    # conv2 pair supergroups: rows 0:48 = (dy,-1), rows 64:112 = (dy,0).
    # Rows 48:64 and 112:128 are zero (engine partition starts must be
    # 32-aligned, so the stacked copies land at 0 and 64).
    W2P = np.zeros((128, 12, 128), np.float32)
    W2S = np.zeros((48, 12, 128), np.float32)
    for d in range(3):                                     # d = dy+1
        for nb in range(NB):
            W2P[0:48, d * 4 + nb] = W2[d * 3 + 0, nb]
            W2P[64:112, d * 4 + nb] = W2[d * 3 + 1, nb]
            W2S[:, d * 4 + nb] = W2[d * 3 + 2, nb]

    IPK = np.concatenate(
        [np.eye(128, dtype=np.float32), -0.01 * np.eye(128, dtype=np.float32)],
        axis=1)  # [128, 256]

    import ml_dtypes
    bf16 = np.dtype(ml_dtypes.bfloat16)
    fp8 = np.dtype(ml_dtypes.float8_e4m3)
    return {"W1D": W1D.astype(fp8), "W2P": W2P.astype(bf16),
            "W2S": W2S.astype(bf16), "IPK": IPK}


def _pack_x(xs):
    """[3,64,64] raw x -> [48, 18, 18] phase layout, zero padded."""
    v = np.asarray(xs, np.float32).reshape(IC, J, P, J, P)
    out = np.zeros((48, 18, 18), np.float32)
    out[:, 1:17, 1:17] = v.transpose(0, 2, 4, 1, 3).reshape(48, J, J)
    return out


def _unpack_a(res):
    """[128, 4, 16, 16] phase layout -> [32, 64, 64]."""
    r = res.reshape(NSUB, P, P, NB, J, J)
    return np.ascontiguousarray(
        r.transpose(3, 0, 4, 1, 5, 2).reshape(NN, H, W))


# ------------------------------------------------------------- device build
def _install_ntff_hook():
    """Re-register the NTFF profile hook this image's antenv lacks."""
    try:
        from antenv.axon_hooks import get_axon_ntff_profile_hook  # noqa: F401
        return
    except ImportError:
        pass
    try:
        import antenv
        mod = types.ModuleType("antenv.axon_hooks")
        _h = [None]
        mod.set_axon_ntff_profile_hook = lambda h: _h.__setitem__(0, h)
        mod.get_axon_ntff_profile_hook = lambda: _h[0]
        sys.modules["antenv.axon_hooks"] = mod
        antenv.axon_hooks = mod
        if "/root/.axon_site" not in sys.path:
            sys.path.insert(0, "/root/.axon_site")
        from trn_agent_boot.trn_boot import _ntff_profile_via_ctypes
        hook = _ntff_profile_via_ctypes('/opt/axon/libaxon_pjrt.so')
        if hook is not None:
            mod.set_axon_ntff_profile_hook(hook)
    except Exception:
        pass


def _build(iters_dev=ITERS_DEV):
    import concourse.tile as tile
    from concourse import bacc, mybir

    f32 = mybir.dt.float32
    f32r = mybir.dt.float32r
    bf16 = mybir.dt.bfloat16
    fp8 = mybir.dt.float8e4
    SU = 32.0                            # u is scaled by SU so fp8 a stays
    TS = THRESH * SU                     # out of the denormal range

    nc = bacc.Bacc(None)
    XL = nc.declare_dram_parameter("XL", [48, 18, 18], f32, isOutput=False)
    W1D = nc.declare_dram_parameter("W1D", [128, 9, 2, 2, 48], fp8, isOutput=False)
    W2P = nc.declare_dram_parameter("W2P", [128, 12, 128], bf16, isOutput=False)
    W2S = nc.declare_dram_parameter("W2S", [48, 12, 128], bf16, isOutput=False)
    IPK = nc.declare_dram_parameter("IPK", [128, 256], f32r, isOutput=False)
    AOUT = nc.declare_dram_parameter("AOUT", [128, 4, J, J], f32, isOutput=True)

    with tile.TileContext(nc) as tc:
        import contextlib
        with contextlib.ExitStack() as ctx:
            sb = ctx.enter_context(tc.tile_pool(name="sb", bufs=1))
            ps = ctx.enter_context(tc.tile_pool(name="ps", bufs=1, space="PSUM"))

            # ---- constants / inputs into SBUF (input + small tensors first,
            # conv1 weights last: b-conv needs only W2P/W2S)
            w1 = sb.tile([128, 9, 2, 2, 48], fp8, tag="w1", name="w1")
            w2p = sb.tile([128, 12, 128], bf16, tag="w2p", name="w2p")
            w2s = sb.tile([128, 12, 128], bf16, tag="w2s", name="w2s")
            ipk = sb.tile([128, 256], f32r, tag="ipk", name="ipk")
            X = sb.tile([48, 18, 18], f32, tag="X", name="X")
            Xr = sb.tile([48, 18, 18], f32r, tag="Xr", name="Xr")
            ones = sb.tile([48, 1], f32r, tag="ones", name="ones")
            ones2 = sb.tile([1, 48], f32r, tag="ones2", name="ones2")
            nc.sync.dma_start(out=X[:], in_=XL[:])
            nc.sync.dma_start(out=ipk[:], in_=IPK[:])
            nc.vector.memset(w2s[32:64, :, :], 0.0)
            nc.vector.memset(w2s[64:128, :, :], 0.0)
            nc.sync.dma_start(out=w2s[0:48, :, :], in_=W2S[:])
            nc.sync.dma_start(out=w2p[:], in_=W2P[:])
            nc.sync.dma_start(out=w1[:], in_=W1D[:])

            Xi = X[:, 1:17, 1:17]                        # [48,16,16] interior
            Xri = Xr[:, 1:17, 1:17]

            # ---- standardization: mean/rstd over the 12288 real values
            nc.vector.memset(Xr[:].bitcast(f32), 0.0)
            nc.vector.memset(ones[:].bitcast(f32), 1.0)
            nc.vector.memset(ones2[:].bitcast(f32), 1.0)
            nc.vector.tensor_copy(Xri, Xi)               # f32 -> f32r round
            sq = sb.tile([48, J, J], f32r, tag="sq", name="sq")
            nc.vector.tensor_mul(sq[:], Xi, Xi)
            psx = ps.tile([1, 256], f32, tag="psx", name="psx")
            psq = ps.tile([1, 256], f32, tag="psq", name="psq")
            nc.tensor.matmul(psx[:], ones[:], Xri, start=True, stop=True)
            nc.tensor.matmul(psq[:], ones[:], sq[:], start=True, stop=True)
            sc = sb.tile([1, 8], f32, tag="sc", name="sc")   # scratch scalars
            nc.vector.reduce_sum(sc[:, 0:1], psx[:], axis=mybir.AxisListType.X)
            nc.vector.reduce_sum(sc[:, 2:3], psq[:], axis=mybir.AxisListType.X)
            n = float(IC * H * W)
            # var = (Sxx - Sx^2/n) / (n-1)
            nc.vector.tensor_mul(sc[:, 4:5], sc[:, 0:1], sc[:, 0:1])   # Sx^2
            nc.vector.tensor_scalar_mul(sc[:, 4:5], sc[:, 4:5], 1.0 / n)
            nc.vector.tensor_sub(sc[:, 4:5], sc[:, 2:3], sc[:, 4:5])
            nc.vector.tensor_scalar_mul(sc[:, 4:5], sc[:, 4:5], 1.0 / (n - 1.0))
            nc.scalar.activation(sc[:, 4:5], sc[:, 4:5],
                                 mybir.ActivationFunctionType.Sqrt)
            nc.vector.reciprocal(sc[:, 4:5], sc[:, 4:5])
            # NEGATED rstd: b-conv reuses conv2 weights (scale -0.01), so feed
            # it -x2 to get +0.01*b into psum.
            nc.vector.tensor_scalar_mul(sc[:, 1:2], sc[:, 4:5], -SU)
            nc.vector.tensor_scalar_mul(sc[:, 0:1], sc[:, 0:1], 1.0 / n)  # mean
            # broadcast mean/-rstd to the 48 partitions via a K=1 matmul
            scr = sb.tile([1, 2], f32r, tag="scr", name="scr")
            nc.vector.tensor_copy(scr[:], sc[:, 0:2])
            psb = ps.tile([48, 2], f32, tag="psb", name="psb")
            nc.tensor.matmul(psb[:], ones2[:], scr[:],
                             start=True, stop=True)
            ms = sb.tile([48, 2], f32, tag="ms", name="ms")
            nc.scalar.activation(ms[:], psb[:],
                                 mybir.ActivationFunctionType.Copy)
            # standardize (negated) straight into the padded f32r conv-input
            nc.vector.tensor_scalar(out=Xri, in0=Xi,
                                    scalar1=ms[:, 0:1], scalar2=ms[:, 1:2],
                                    op0=mybir.AluOpType.subtract,
                                    op1=mybir.AluOpType.mult)

            # ---- PE warmup: dummy matmuls during the std/DMA wait keep the
            # tensor clock ramping so the b-conv runs at full speed
            for _ in range(12):
                nc.tensor.matmul(psx[:], ones[:], sq[:], start=True, stop=True)

            # ---- stacked X tile (tap pairs in K) for the b-conv
            XS = sb.tile([128, 18, 18], bf16, tag="XS", name="XS")
            nc.vector.memset(XS[:], 0.0)
            nc.scalar.activation(XS[0:48, :, :], Xr[:],
                                 mybir.ActivationFunctionType.Copy)
            nc.scalar.activation(XS[64:112, :, 0:17], Xr[:, :, 1:18],
                                 mybir.ActivationFunctionType.Copy)

            # ---- b-conv: psum_u <- u_1 = 0.01*b
            pu = [ps.tile([128, J, J], f32, tag=f"pu{k}", name=f"pu{k}")
                  for k in range(NB)]
            for nb in range(NB):
                for d in range(3):                       # d = dy+1
                    nc.tensor.matmul(pu[nb][:], w2s[:, d * 4 + nb, :],
                                     XS[:, d:16 + d, 2:18],
                                     start=(d == 0), stop=False)
                for d in range(3):
                    nc.tensor.matmul(pu[nb][:], w2p[:, d * 4 + nb, :],
                                     XS[:, d:16 + d, 0:16],
                                     start=False, stop=(d == 2))

            # ---- SBUF state
            b01 = sb.tile([128, NB, J, J], f32r, tag="b01", name="b01")
            C = sb.tile([128, NB, J, J], f32r, tag="C", name="C")
            Csc = sb.tile([128, NB, J, J], f32r, tag="Csc", name="Csc")
            T = sb.tile([128, NB, J, J], f32r, tag="T", name="T")
            A = sb.tile([128, 2, 2, 24, 20], fp8, tag="A", name="A")
            R1 = sb.tile([128, 18, 18], bf16, tag="R1", name="R1")
            nc.vector.memset(A[:], 0.0)
            nc.vector.memset(R1[:], 0.0)
            for nb in range(NB):
                nc.scalar.activation(b01[:, nb, :, :], pu[nb][:],
                                     mybir.ActivationFunctionType.Copy)

            # recon stored 20 cols/row (cols 18,19 junk) so conv1 rhs windows
            # are single fully-contiguous 360-col runs of the flattened A grid
            recon = ps.tile([48, 18, 20], f32, tag="recon", name="recon")

            # ---- LCA iterations (u stays in PSUM)
            for it in range(iters_dev):
                for nb in range(NB):
                    cv = C[:, nb, :, :]
                    nc.vector.tensor_scalar(out=cv, in0=pu[nb][:],
                                            scalar1=TS, scalar2=-TS,
                                            op0=mybir.AluOpType.min,
                                            op1=mybir.AluOpType.max)
                    nc.vector.tensor_sub(A[:, nb // 2, nb % 2, 2:18, 2:18],
                                         pu[nb][:], cv)
                # conv1: recon = D a. fp8 DoubleRow: the 2 members of each
                # nb-pair are the 2 K-tiles (rhs k-stride 480, %16 == 0)
                first = True
                for p in range(2):
                    Af = A[:, p, :, :, :].rearrange("p a b c -> p a (b c)")
                    for g in range(9):
                        gy, gx = g // 3, g % 3           # 0..2 = offset+1
                        o = gy * 20 + gx
                        nc.tensor.matmul(recon[:].opt(), w1[:, g, p, :, :],
                                         Af[:, :, o:o + 360],
                                         start=first,
                                         stop=(p == 1 and g == 8),
                                         perf_mode=mybir.MatmulPerfMode.DoubleRow)
                        first = False
                # R-stack copies (scalar + vector in parallel; overlapped by
                # the ident matmuls)
                nc.scalar.activation(R1[0:48, :, :], recon[:, :, 0:18],
                                     mybir.ActivationFunctionType.Copy)
                nc.vector.tensor_copy(R1[64:112, :, 0:17], recon[:, :, 1:18])
                # T = b01 - 0.01*C on gpsimd (idle engine, SBUF-only)
                for nb in range(NB):
                    nc.gpsimd.tensor_scalar_mul(Csc[:, nb, :, :],
                                                C[:, nb, :, :], 0.01)
                    nc.gpsimd.tensor_sub(T[:, nb, :, :], b01[:, nb, :, :],
                                         Csc[:, nb, :, :])
                # ident: u += T
                for nb in range(NB):
                    nc.tensor.matmul(pu[nb][:], ipk[:, 0:128],
                                     T[:, nb, :, :], start=False, stop=False)
                # conv2: singles first (need only the R1[0:48] copy), pairs
                # second -- the second R-stack copy hides under the singles
                for nb in range(NB):
                    for d in range(3):
                        nc.tensor.matmul(pu[nb][:], w2s[:, d * 4 + nb, :],
                                         R1[:, d:16 + d, 2:18],
                                         start=False, stop=False)
                for nb in range(NB):
                    for d in range(3):
                        nc.tensor.matmul(pu[nb][:], w2p[:, d * 4 + nb, :],
                                         R1[:, d:16 + d, 0:16],
                                         start=False, stop=(d == 2))

            # ---- final a_10 = (u_9' - clip(u_9'))/SU  (per-nb, DMA overlapped)
            asub = sb.tile([128, NB, J, J], f32, tag="asub", name="asub")
            aout = sb.tile([128, NB, J, J], f32, tag="aout", name="aout")
            for nb in range(NB):
                cv = C[:, nb, :, :]
                nc.vector.tensor_scalar(out=cv, in0=pu[nb][:],
                                        scalar1=TS, scalar2=-TS,
                                        op0=mybir.AluOpType.min,
                                        op1=mybir.AluOpType.max)
                nc.vector.tensor_sub(asub[:, nb, :, :], pu[nb][:], cv)
                nc.scalar.activation(aout[:, nb, :, :], asub[:, nb, :, :],
                                     mybir.ActivationFunctionType.Copy,
                                     scale=1.0 / SU)
                nc.sync.dma_start(out=AOUT[:, nb, :, :], in_=aout[:, nb, :, :])

    nc.finalize()
    return nc


# ---------------------------------------------------------------- interface
def kernel(x, D, _trace=False, _iters_dev=ITERS_DEV):
    from concourse.bass_utils import run_bass_kernel_spmd

    x = np.asarray(x, np.float32)
    D = np.asarray(D, np.float32)

    key = ("nc", _iters_dev)
    if key not in _CACHE:
        _CACHE[key] = _build(_iters_dev)
    nc = _CACHE[key]

    wk = ("wts", D.tobytes()[:64])
    if "wts" not in _CACHE or _CACHE.get("wts_id") != wk:
        _CACHE["wts"] = _host_pack(D)
        _CACHE["wts_id"] = wk
    wts = _CACHE["wts"]

    core_ids = list(range(B))
    in_maps = []
    for b in range(B):
        in_maps.append({
            "XL": _pack_x(x[b, :, 0]),
            "W1D": wts["W1D"],
            "W2P": wts["W2P"],
            "W2S": wts["W2S"],
            "IPK": wts["IPK"],
        })

    if _trace:
        _install_ntff_hook()
    res = run_bass_kernel_spmd(nc, in_maps, core_ids, trace=_trace)

    out = np.empty((B, NN, 1, H, W), np.float32)
    for b in range(B):
        out[b, :, 0] = _unpack_a(res.results[b]["AOUT"])
    if _trace:
        kernel._last_exec_ns = res.exec_time_ns
    return out


# revision 24
# speedup vs baseline: 2.6045x; 2.6045x over previous
"""LCAConv (locally competitive algorithm, convolutional sparse coding) on
8 trn2 NeuronCores — data-parallel over batch (1 sample per core).

Math (matches the jax reference):
  x2   = standardize(x)                       per-sample, ddof=1, eps 1e-12
  b    = conv(x2, D, pad=3)                   input drive [32,64,64]
  u_1  = 0.01*b;  a_t = soft_threshold(u_{t-1}, 0.1)
  u_t  = u_{t-1} + 0.01*b - 0.01*clip(u_{t-1}, +-0.1) - 0.01*conv(a_t, G, pad=6)
  out  = a_10 = ST(u_9)   ->  8 Gram-conv iterations on device.

The Gram conv is factorized: conv(a, G) == conv(conv_T(a, D), D), cutting
tensor-engine work ~2.6x vs streaming the 13x13 Gram tensor directly.

Device mapping: 4x4 phase-packed layout. Pixel (y,x) = (4jy+ty, 4jx+tx).
u/a/b live as [128 = 8n x 16ph, 4 n-blocks x 16x16 j-grid]; each LCA
iteration is:
  conv1 (recon = D a):   9 tap-groups x 4 K-chunks, N=324  -> psum [48,18,18]
  R-stack: 2 scalar copies pack tap-pairs into K=96
  ident:  u += b01 - 0.01*clip(u)  (2 matmuls x 4 blocks)
  conv2 (u -= 0.01 D^T recon): 6 supergroups x 4 blocks, N=256
u is accumulated and kept resident in PSUM across all iterations. Matmuls
run in float32r (1 col/cycle, ~1e-4 operand rounding).
"""
import os
import sys
import types
import numpy as np

# ---------------------------------------------------------------- constants
NN, IC, K7 = 32, 3, 7                  # neurons, in-channels, kernel
H = W = 64
P = 4                                  # phase packing
J = H // P                             # 16 j-grid
NB = 4                                 # n-blocks (32/8)
NSUB = NN // NB                        # 8 neurons per block
PH = P * P                             # 16 phases
THRESH, TAU, LCA_ITERS = 0.1, 100.0, 10
ITERS_DEV = LCA_ITERS - 2              # first iter folded into b, last is ST only
B = 8                                  # batch == n cores

_CACHE = {}


# ------------------------------------------------------------- host packing
def _pack_w1(D2):
    """conv1 (recon = D a) lhsT tiles [9, NB, 128, 48]."""
    W1 = np.zeros((9, NB, 128, 48), np.float64)
    ns_, sy_, sx_ = np.meshgrid(np.arange(NSUB), np.arange(P), np.arange(P),
                                indexing='ij')
    c_, ty_, tx_ = np.meshgrid(np.arange(IC), np.arange(P), np.arange(P),
                               indexing='ij')
    for gy in (-1, 0, 1):
        for gx in (-1, 0, 1):
            g = (gy + 1) * 3 + (gx + 1)
            for nb in range(NB):
                ky = ty_.reshape(1, -1) - sy_.reshape(-1, 1) - 4 * gy + 3
                kx = tx_.reshape(1, -1) - sx_.reshape(-1, 1) - 4 * gx + 3
                ok = (ky >= 0) & (ky < 7) & (kx >= 0) & (kx < 7)
                n_full = (nb * NSUB + ns_).reshape(-1, 1)
                c_full = c_.reshape(1, -1)
                vals = D2[n_full, c_full, np.clip(ky, 0, 6), np.clip(kx, 0, 6)]
                W1[g, nb] = np.where(ok, vals, 0.0)
    return W1


def _pack_w2(D2, scale):
    """conv2 / b-conv lhsT tiles [9, NB, 48, 128], value scale*D2."""
    W2 = np.zeros((9, NB, 48, 128), np.float64)
    c_, sy_, sx_ = np.meshgrid(np.arange(IC), np.arange(P), np.arange(P),
                               indexing='ij')
    ns_, ty_, tx_ = np.meshgrid(np.arange(NSUB), np.arange(P), np.arange(P),
                                indexing='ij')
    for gy in (-1, 0, 1):
        for gx in (-1, 0, 1):
            g = (gy + 1) * 3 + (gx + 1)
            for nb in range(NB):
                ky = 4 * gy + sy_.reshape(-1, 1) - ty_.reshape(1, -1) + 3
                kx = 4 * gx + sx_.reshape(-1, 1) - tx_.reshape(1, -1) + 3
                ok = (ky >= 0) & (ky < 7) & (kx >= 0) & (kx < 7)
                n_full = (nb * NSUB + ns_).reshape(1, -1)
                c_full = c_.reshape(-1, 1)
                vals = D2[n_full, c_full, np.clip(ky, 0, 6), np.clip(kx, 0, 6)]
                W2[g, nb] = np.where(ok, scale * vals, 0.0)
    return W2


def _host_pack(D):
    """Everything derived from D only (weights)."""
    D2 = np.asarray(D, np.float64).reshape(NN, IC, K7, K7)
    W1 = _pack_w1(D2)                                      # [9,4,128,48]
    W2 = _pack_w2(D2, -0.01)                               # [9,4,48,128]

    # conv1 lhsT, fp8 DoubleRow: [128, g, pair, member, 48], nb = pair*2+member
    W1D = np.ascontiguousarray(
        W1.reshape(9, 2, 2, 128, 48).transpose(3, 0, 1, 2, 4))

    # conv2 pair supergroups: rows 0:48 = (dy,-1), rows 64:112 = (dy,0).
    # Rows 48:64 and 112:128 are zero (engine partition starts must be
    # 32-aligned, so the stacked copies land at 0 and 64).
    W2P = np.zeros((128, 12, 128), np.float32)
    W2S = np.zeros((48, 12, 128), np.float32)
    for d in range(3):                                     # d = dy+1
        for nb in range(NB):
            W2P[0:48, d * 4 + nb] = W2[d * 3 + 0, nb]
            W2P[64:112, d * 4 + nb] = W2[d * 3 + 1, nb]
            W2S[:, d * 4 + nb] = W2[d * 3 + 2, nb]

    IPK = np.concatenate(
        [np.eye(128, dtype=np.float32), -0.01 * np.eye(128, dtype=np.float32)],
        axis=1)  # [128, 256]

    import ml_dtypes
    bf16 = np.dtype(ml_dtypes.bfloat16)
    fp8 = np.dtype(ml_dtypes.float8_e4m3)
    return {"W1D": W1D.astype(fp8), "W2P": W2P.astype(bf16),
            "W2S": W2S.astype(bf16), "IPK": IPK}


def _pack_x(xs):
    """[3,64,64] raw x -> [48, 18, 18] phase layout, zero padded."""
    v = np.asarray(xs, np.float32).reshape(IC, J, P, J, P)
    out = np.zeros((48, 18, 18), np.float32)
    out[:, 1:17, 1:17] = v.transpose(0, 2, 4, 1, 3).reshape(48, J, J)
    return out


def _unpack_a(res):
    """[128, 4, 16, 16] phase layout -> [32, 64, 64]."""
    r = res.reshape(NSUB, P, P, NB, J, J)
    return np.ascontiguousarray(
        r.transpose(3, 0, 4, 1, 5, 2).reshape(NN, H, W))


# ------------------------------------------------------------- device build
def _install_ntff_hook():
    """Re-register the NTFF profile hook this image's antenv lacks."""
    try:
        from antenv.axon_hooks import get_axon_ntff_profile_hook  # noqa: F401
        return
    except ImportError:
        pass
    try:
        import antenv
        mod = types.ModuleType("antenv.axon_hooks")
        _h = [None]
        mod.set_axon_ntff_profile_hook = lambda h: _h.__setitem__(0, h)
        mod.get_axon_ntff_profile_hook = lambda: _h[0]
        sys.modules["antenv.axon_hooks"] = mod
        antenv.axon_hooks = mod
        if "/root/.axon_site" not in sys.path:
            sys.path.insert(0, "/root/.axon_site")
        from trn_agent_boot.trn_boot import _ntff_profile_via_ctypes
        hook = _ntff_profile_via_ctypes('/opt/axon/libaxon_pjrt.so')
        if hook is not None:
            mod.set_axon_ntff_profile_hook(hook)
    except Exception:
        pass


def _build(iters_dev=ITERS_DEV):
    import concourse.tile as tile
    from concourse import bacc, mybir

    f32 = mybir.dt.float32
    f32r = mybir.dt.float32r
    bf16 = mybir.dt.bfloat16
    fp8 = mybir.dt.float8e4
    SU = 32.0                            # u is scaled by SU so fp8 a stays
    TS = THRESH * SU                     # out of the denormal range

    nc = bacc.Bacc(None)
    XL = nc.declare_dram_parameter("XL", [48, 18, 18], f32, isOutput=False)
    W1D = nc.declare_dram_parameter("W1D", [128, 9, 2, 2, 48], fp8, isOutput=False)
    W2P = nc.declare_dram_parameter("W2P", [128, 12, 128], bf16, isOutput=False)
    W2S = nc.declare_dram_parameter("W2S", [48, 12, 128], bf16, isOutput=False)
    IPK = nc.declare_dram_parameter("IPK", [128, 256], f32r, isOutput=False)
    AOUT = nc.declare_dram_parameter("AOUT", [128, 4, J, J], f32, isOutput=True)

    with tile.TileContext(nc) as tc:
        import contextlib
        with contextlib.ExitStack() as ctx:
            sb = ctx.enter_context(tc.tile_pool(name="sb", bufs=1))
            ps = ctx.enter_context(tc.tile_pool(name="ps", bufs=1, space="PSUM"))

            # ---- constants / inputs into SBUF (input + small tensors first,
            # conv1 weights last: b-conv needs only W2P/W2S)
            w1 = sb.tile([128, 9, 2, 2, 48], fp8, tag="w1", name="w1")
            w2p = sb.tile([128, 12, 128], bf16, tag="w2p", name="w2p")
            w2s = sb.tile([128, 12, 128], bf16, tag="w2s", name="w2s")
            ipk = sb.tile([128, 256], f32r, tag="ipk", name="ipk")
            X = sb.tile([48, 18, 18], f32, tag="X", name="X")
            Xr = sb.tile([48, 18, 18], f32r, tag="Xr", name="Xr")
            ones = sb.tile([48, 1], f32r, tag="ones", name="ones")
            ones2 = sb.tile([1, 48], f32r, tag="ones2", name="ones2")
            nc.sync.dma_start(out=X[:], in_=XL[:])
            nc.sync.dma_start(out=ipk[:], in_=IPK[:])
            nc.vector.memset(w2s[32:64, :, :], 0.0)
            nc.vector.memset(w2s[64:128, :, :], 0.0)
            nc.sync.dma_start(out=w2s[0:48, :, :], in_=W2S[:])
            nc.sync.dma_start(out=w2p[:], in_=W2P[:])
            nc.sync.dma_start(out=w1[:], in_=W1D[:])

            Xi = X[:, 1:17, 1:17]                        # [48,16,16] interior
            Xri = Xr[:, 1:17, 1:17]

            # ---- standardization: mean/rstd over the 12288 real values
            nc.vector.memset(Xr[:].bitcast(f32), 0.0)
            nc.vector.memset(ones[:].bitcast(f32), 1.0)
            nc.vector.memset(ones2[:].bitcast(f32), 1.0)
            nc.vector.tensor_copy(Xri, Xi)               # f32 -> f32r round
            sq = sb.tile([48, J, J], f32r, tag="sq", name="sq")
            nc.vector.tensor_mul(sq[:], Xi, Xi)
            psx = ps.tile([1, 256], f32, tag="psx", name="psx")
            psq = ps.tile([1, 256], f32, tag="psq", name="psq")
            nc.tensor.matmul(psx[:], ones[:], Xri, start=True, stop=True)
            nc.tensor.matmul(psq[:], ones[:], sq[:], start=True, stop=True)
            sc = sb.tile([1, 8], f32, tag="sc", name="sc")   # scratch scalars
            nc.vector.reduce_sum(sc[:, 0:1], psx[:], axis=mybir.AxisListType.X)
            nc.vector.reduce_sum(sc[:, 2:3], psq[:], axis=mybir.AxisListType.X)
            n = float(IC * H * W)
            # var = (Sxx - Sx^2/n) / (n-1)
            nc.vector.tensor_mul(sc[:, 4:5], sc[:, 0:1], sc[:, 0:1])   # Sx^2
            nc.vector.tensor_scalar_mul(sc[:, 4:5], sc[:, 4:5], 1.0 / n)
            nc.vector.tensor_sub(sc[:, 4:5], sc[:, 2:3], sc[:, 4:5])
            nc.vector.tensor_scalar_mul(sc[:, 4:5], sc[:, 4:5], 1.0 / (n - 1.0))
            nc.scalar.activation(sc[:, 4:5], sc[:, 4:5],
                                 mybir.ActivationFunctionType.Sqrt)
            nc.vector.reciprocal(sc[:, 4:5], sc[:, 4:5])
            # NEGATED rstd: b-conv reuses conv2 weights (scale -0.01), so feed
            # it -x2 to get +0.01*b into psum.
            nc.vector.tensor_scalar_mul(sc[:, 1:2], sc[:, 4:5], -SU)
            nc.vector.tensor_scalar_mul(sc[:, 0:1], sc[:, 0:1], 1.0 / n)  # mean
            # broadcast mean/-rstd to the 48 partitions via a K=1 matmul
            scr = sb.tile([1, 2], f32r, tag="scr", name="scr")
            nc.vector.tensor_copy(scr[:], sc[:, 0:2])
            psb = ps.tile([48, 2], f32, tag="psb", name="psb")
            nc.tensor.matmul(psb[:], ones2[:], scr[:],
                             start=True, stop=True)
            ms = sb.tile([48, 2], f32, tag="ms", name="ms")
            nc.scalar.activation(ms[:], psb[:],
                                 mybir.ActivationFunctionType.Copy)
            # standardize (negated) straight into the padded f32r conv-input
            nc.vector.tensor_scalar(out=Xri, in0=Xi,
                                    scalar1=ms[:, 0:1], scalar2=ms[:, 1:2],
                                    op0=mybir.AluOpType.subtract,
                                    op1=mybir.AluOpType.mult)

            # ---- PE warmup: dummy matmuls during the std/DMA wait keep the
            # tensor clock ramping so the b-conv runs at full speed
            for _ in range(12):
                nc.tensor.matmul(psx[:], ones[:], sq[:], start=True, stop=True)

            # ---- stacked X tile (tap pairs in K) for the b-conv
            XS = sb.tile([128, 18, 18], bf16, tag="XS", name="XS")
            nc.vector.memset(XS[:], 0.0)
            nc.scalar.activation(XS[0:48, :, :], Xr[:],
                                 mybir.ActivationFunctionType.Copy)
            nc.scalar.activation(XS[64:112, :, 0:17], Xr[:, :, 1:18],
                                 mybir.ActivationFunctionType.Copy)

            # ---- b-conv: psum_u <- u_1 = 0.01*b
            pu = [ps.tile([128, J, J], f32, tag=f"pu{k}", name=f"pu{k}")
                  for k in range(NB)]
            for nb in range(NB):
                for d in range(3):                       # d = dy+1
                    nc.tensor.matmul(pu[nb][:], w2s[:, d * 4 + nb, :],
                                     XS[:, d:16 + d, 2:18],
                                     start=(d == 0), stop=False)
                for d in range(3):
                    nc.tensor.matmul(pu[nb][:], w2p[:, d * 4 + nb, :],
                                     XS[:, d:16 + d, 0:16],
                                     start=False, stop=(d == 2))

            # ---- SBUF state
            b01 = sb.tile([128, NB, J, J], f32r, tag="b01", name="b01")
            C = sb.tile([128, NB, J, J], f32r, tag="C", name="C")
            A = sb.tile([128, 2, 2, 24, 20], fp8, tag="A", name="A")
            R1 = sb.tile([128, 18, 18], bf16, tag="R1", name="R1")
            nc.vector.memset(A[:], 0.0)
            nc.vector.memset(R1[:], 0.0)
            for nb in range(NB):
                nc.scalar.activation(b01[:, nb, :, :], pu[nb][:],
                                     mybir.ActivationFunctionType.Copy)

            # recon stored 20 cols/row (cols 18,19 junk) so conv1 rhs windows
            # are single fully-contiguous 360-col runs of the flattened A grid
            recon = ps.tile([48, 18, 20], f32, tag="recon", name="recon")

            # ---- LCA iterations (u stays in PSUM)
            for it in range(iters_dev):
                for nb in range(NB):
                    cv = C[:, nb, :, :]
                    nc.vector.tensor_scalar(out=cv, in0=pu[nb][:],
                                            scalar1=TS, scalar2=-TS,
                                            op0=mybir.AluOpType.min,
                                            op1=mybir.AluOpType.max)
                    nc.vector.tensor_sub(A[:, nb // 2, nb % 2, 2:18, 2:18],
                                         pu[nb][:], cv)
                # conv1: recon = D a. fp8 DoubleRow: the 2 members of each
                # nb-pair are the 2 K-tiles (rhs k-stride 480, %16 == 0)
                first = True
                for p in range(2):
                    Af = A[:, p, :, :, :].rearrange("p a b c -> p a (b c)")
                    for g in range(9):
                        gy, gx = g // 3, g % 3           # 0..2 = offset+1
                        o = gy * 20 + gx
                        nc.tensor.matmul(recon[:].opt(), w1[:, g, p, :, :],
                                         Af[:, :, o:o + 360],
                                         start=first,
                                         stop=(p == 1 and g == 8),
                                         perf_mode=mybir.MatmulPerfMode.DoubleRow)
                        first = False
                # R-stack copies (scalar + vector in parallel; overlapped by
                # the ident matmuls)
                nc.scalar.activation(R1[0:48, :, :], recon[:, :, 0:18],
                                     mybir.ActivationFunctionType.Copy)
                nc.vector.tensor_copy(R1[64:112, :, 0:17], recon[:, :, 1:18])
                # ident: u += b01 - 0.01*C
                for nb in range(NB):
                    nc.tensor.matmul(pu[nb][:], ipk[:, 0:128],
                                     b01[:, nb, :, :], start=False, stop=False)
                    nc.tensor.matmul(pu[nb][:], ipk[:, 128:256],
                                     C[:, nb, :, :], start=False, stop=False)
                # conv2: singles first (need only the R1[0:48] copy), pairs
                # second -- the second R-stack copy hides under the singles
                for nb in range(NB):
                    for d in range(3):
                        nc.tensor.matmul(pu[nb][:], w2s[:, d * 4 + nb, :],
                                         R1[:, d:16 + d, 2:18],
                                         start=False, stop=False)
                for nb in range(NB):
                    for d in range(3):
                        nc.tensor.matmul(pu[nb][:], w2p[:, d * 4 + nb, :],
                                         R1[:, d:16 + d, 0:16],
                                         start=False, stop=(d == 2))

            # ---- final a_10 = (u_9' - clip(u_9'))/SU  (per-nb, DMA overlapped)
            asub = sb.tile([128, NB, J, J], f32, tag="asub", name="asub")
            aout = sb.tile([128, NB, J, J], f32, tag="aout", name="aout")
            for nb in range(NB):
                cv = C[:, nb, :, :]
                nc.vector.tensor_scalar(out=cv, in0=pu[nb][:],
                                        scalar1=TS, scalar2=-TS,
                                        op0=mybir.AluOpType.min,
                                        op1=mybir.AluOpType.max)
                nc.vector.tensor_sub(asub[:, nb, :, :], pu[nb][:], cv)
                nc.scalar.activation(aout[:, nb, :, :], asub[:, nb, :, :],
                                     mybir.ActivationFunctionType.Copy,
                                     scale=1.0 / SU)
                nc.sync.dma_start(out=AOUT[:, nb, :, :], in_=aout[:, nb, :, :])

    nc.finalize()
    return nc


# ---------------------------------------------------------------- interface
def kernel(x, D, _trace=False, _iters_dev=ITERS_DEV):
    from concourse.bass_utils import run_bass_kernel_spmd

    x = np.asarray(x, np.float32)
    D = np.asarray(D, np.float32)

    key = ("nc", _iters_dev)
    if key not in _CACHE:
        _CACHE[key] = _build(_iters_dev)
    nc = _CACHE[key]

    wk = ("wts", D.tobytes()[:64])
    if "wts" not in _CACHE or _CACHE.get("wts_id") != wk:
        _CACHE["wts"] = _host_pack(D)
        _CACHE["wts_id"] = wk
    wts = _CACHE["wts"]

    core_ids = list(range(B))
    in_maps = []
    for b in range(B):
        in_maps.append({
            "XL": _pack_x(x[b, :, 0]),
            "W1D": wts["W1D"],
            "W2P": wts["W2P"],
            "W2S": wts["W2S"],
            "IPK": wts["IPK"],
        })

    if _trace:
        _install_ntff_hook()
    res = run_bass_kernel_spmd(nc, in_maps, core_ids, trace=_trace)

    out = np.empty((B, NN, 1, H, W), np.float32)
    for b in range(B):
        out[b, :, 0] = _unpack_a(res.results[b]["AOUT"])
    if _trace:
        kernel._last_exec_ns = res.exec_time_ns
    return out
